# revision 2
# baseline (speedup 1.0000x reference)
"""Trainium2 Bass kernel for nn_Dense_25202868092933.

Computes: outs = einsum('bit,oi->bot', inputs, W); then a 64-step
leaky-integrate-and-fire neuron scan with hard reset:
    mem = mult*mem + scale*outs[..., t];  spk = (mem <= -0.5);  mem *= (1-spk)

Strategy (8 NeuronCores, no cross-core communication):
  - Tensor-parallel over out_features: each core owns 512 rows of W,
    the scale/mult vectors for those rows, and the scan state.
  - scale is folded into W on the host (W' = diag(scale) @ W), so the
    matmul directly produces scale*outs.
  - The matmul runs as a 3-pass bf16 hi/lo split (x_hi@W_hi + x_lo@W_hi
    + x_hi@W_lo), which reproduces fp32 accuracy to ~1.5e-6 abs on the
    pre-scan activations (measured: zero spike flips) at bf16 PE rate.
  - bt = t*64+b is the matmul moving dimension; blocks of 256 bt (4
    timesteps) pipeline matmul (PE) -> psum copy (ACT) -> scan (DVE)
    -> spike DMA, so the scan hides under the next block's matmuls.
  - The kernel emits "no-spike" indicators (mem > thr); the host
    computes spikes = 1 - nspk (exact complement of float 0/1).
"""

import os
import sys

for _p in ("/opt/trn_rl_repo", "/root/.axon_site", "/root/.axon_site/_ro/trn_rl_repo",
           "/root/.axon_site/_ro/pypackages"):
    if os.path.isdir(_p) and _p not in sys.path:
        sys.path.append(_p)

from contextlib import ExitStack

import ml_dtypes
import numpy as np

import concourse.bass as bass  # noqa: F401  (registers engine classes)
import concourse.mybir as mybir
import concourse.tile as tile
from concourse import bacc
from concourse.bass_utils import run_bass_kernel_spmd

# Problem constants
B, F_IN, F_OUT, T = 64, 4096, 4096, 64
KVCO = 5.0e8
KPD = 1.0 / np.pi
TIMESTEP = 1e-9
SCALING = KVCO * KPD * TIMESTEP / 2.0
THRESHOLD = -0.5

NCORES = 8
FS = F_OUT // NCORES      # 512 out-features per core
P = 128                   # partitions
M = FS // P               # 4 feature tiles per core
KO = F_IN // P            # 32 contraction tiles
BT = B * T                # 4096 moving dim (t-major: bt = t*B + b)
NJ = 256                  # bt per block
NB = BT // NJ             # 16 blocks
TL = NJ // B              # 4 timesteps per block

_dt = mybir.dt

_CACHE = {}


def _build_program():
    """Emit the SPMD Tile program (identical on all 8 cores)."""
    nc = bacc.Bacc("TRN2", target_bir_lowering=False, debug=False)

    xhi_d = nc.dram_tensor("xhi", [NB, P, KO * NJ], _dt.bfloat16, kind="ExternalInput").ap()
    xlo_d = nc.dram_tensor("xlo", [NB, P, KO * NJ], _dt.bfloat16, kind="ExternalInput").ap()
    whi_d = nc.dram_tensor("whi", [P, M * KO * P], _dt.bfloat16, kind="ExternalInput").ap()
    wlo_d = nc.dram_tensor("wlo", [P, M * KO * P], _dt.bfloat16, kind="ExternalInput").ap()
    mlt_d = nc.dram_tensor("multf", [P, M * B], _dt.float32, kind="ExternalInput").ap()
    out_d = nc.dram_tensor("nspk", [P, M * NB * NJ], _dt.float32, kind="ExternalOutput").ap()
    out4 = out_d.rearrange("p (m nb j) -> p m nb j", m=M, nb=NB)

    with tile.TileContext(nc) as tc, ExitStack() as ctx:
        wpool = ctx.enter_context(tc.tile_pool(name="wpool", bufs=1))
        cpool = ctx.enter_context(tc.tile_pool(name="cpool", bufs=1))
        xpool = ctx.enter_context(tc.tile_pool(name="xpool", bufs=2))
        ypool = ctx.enter_context(tc.tile_pool(name="ypool", bufs=2))
        npool = ctx.enter_context(tc.tile_pool(name="npool", bufs=2))
        spool = ctx.enter_context(tc.tile_pool(name="spool", bufs=1))
        tpool = ctx.enter_context(tc.tile_pool(name="tpool", bufs=2))
        pspool = ctx.enter_context(tc.tile_pool(name="pspool", bufs=8, space="PSUM"))

        w_hi = wpool.tile([P, M, KO, P], _dt.bfloat16, tag="whi", name="w_hi")
        nc.sync.dma_start(w_hi[:], whi_d.rearrange("p (m k c) -> p m k c", m=M, k=KO))
        w_lo = wpool.tile([P, M, KO, P], _dt.bfloat16, tag="wlo", name="w_lo")
        nc.sync.dma_start(w_lo[:], wlo_d.rearrange("p (m k c) -> p m k c", m=M, k=KO))
        mlt = cpool.tile([P, M, B], _dt.float32, tag="mlt", name="mlt")
        nc.sync.dma_start(mlt[:], mlt_d.rearrange("p (m b) -> p m b", m=M))

        mem = spool.tile([P, M, B], _dt.float32, tag="mem", name="mem")
        nc.vector.memset(mem[:], 0.0)

        for nb in range(NB):
            xh = xpool.tile([P, KO, NJ], _dt.bfloat16, tag="xh", name="xh")
            nc.sync.dma_start(xh[:], xhi_d[nb].rearrange("p (k j) -> p k j", k=KO))
            xl = xpool.tile([P, KO, NJ], _dt.bfloat16, tag="xl", name="xl")
            nc.sync.dma_start(xl[:], xlo_d[nb].rearrange("p (k j) -> p k j", k=KO))

            y = ypool.tile([P, M, TL, B], _dt.float32, tag="y", name="y")
            for m in range(M):
                ps = pspool.tile([P, TL, B], _dt.float32, tag="ps", name="ps")
                for ko in range(KO):
                    # weight-reuse-friendly order: whi used twice in a row
                    nc.tensor.matmul(ps[:], w_hi[:, m, ko, :], xh[:, ko, :],
                                     start=(ko == 0), stop=False)
                    nc.tensor.matmul(ps[:], w_hi[:, m, ko, :], xl[:, ko, :],
                                     start=False, stop=False)
                    nc.tensor.matmul(ps[:], w_lo[:, m, ko, :], xh[:, ko, :],
                                     start=False, stop=(ko == KO - 1))
                nc.scalar.copy(y[:, m, :, :], ps[:])

            nsp = npool.tile([P, M, TL, B], _dt.float32, tag="nsp", name="nsp")
            for tl in range(TL):
                tmp = tpool.tile([P, M, B], _dt.float32, tag="tmp", name="tmp")
                nc.vector.tensor_mul(tmp[:], mem[:], mlt[:])
                nc.vector.tensor_add(mem[:], tmp[:], y[:, :, tl, :])
                nc.vector.tensor_scalar(nsp[:, :, tl, :], mem[:], THRESHOLD, None,
                                        mybir.AluOpType.is_gt)
                nc.vector.tensor_mul(mem[:], mem[:], nsp[:, :, tl, :])

            nc.sync.dma_start(out4[:, :, nb, :],
                              nsp[:].rearrange("p m tl b -> p m (tl b)"))

    nc.compile()
    return nc


def _prep_inputs(inputs: np.ndarray, W: np.ndarray):
    """Host-side preprocessing: neuron constants, scale folding, hi/lo
    bf16 split, and per-core data layouts."""
    f32 = np.float32
    rs = W.sum(axis=1, dtype=np.float64).astype(f32)
    ncst = np.maximum((f32(1.0) + rs) / f32(SCALING), f32(0.0)).astype(f32)
    scale = (f32(-1.0) / (f32(1.0) + ncst)).astype(f32)
    mult = ((ncst - f32(1.0)) / (ncst + f32(1.0))).astype(f32)

    Ws = (W * scale[:, None]).astype(f32)          # fold scale into W rows
    wT = np.ascontiguousarray(Ws.T)                # [F_IN, F_OUT]
    wT_hi = wT.astype(ml_dtypes.bfloat16)
    wT_lo = (wT - wT_hi.astype(f32)).astype(ml_dtypes.bfloat16)

    # x_flat[i, t*B + b] = inputs[b, i, t]
    x_flat = np.ascontiguousarray(inputs.transpose(1, 2, 0)).reshape(F_IN, BT)
    x_hi = x_flat.astype(ml_dtypes.bfloat16)
    x_lo = (x_flat - x_hi.astype(f32)).astype(ml_dtypes.bfloat16)

    def xblocks(xa):
        # [F_IN, BT] -> [NB, P, KO*NJ]; per-partition contiguous 16 KB
        return np.ascontiguousarray(
            xa.reshape(KO, P, NB, NJ).transpose(2, 1, 0, 3)).reshape(NB, P, KO * NJ)

    xh_b = xblocks(x_hi)
    xl_b = xblocks(x_lo)

    in_maps = []
    for c in range(NCORES):
        sl = slice(c * FS, (c + 1) * FS)

        def wlayout(wa):
            # [F_IN, FS] -> [P, M*KO*P]: w[p, m, ko, cc] = wT[ko*P+p, m*P+cc]
            return np.ascontiguousarray(
                wa[:, sl].reshape(KO, P, M, P).transpose(1, 2, 0, 3)).reshape(P, M * KO * P)

        mc = mult[sl].reshape(M, P).T              # [P, M]
        multf = np.ascontiguousarray(
            np.broadcast_to(mc[:, :, None], (P, M, B))).reshape(P, M * B).astype(f32)

        in_maps.append({
            "xhi": xh_b, "xlo": xl_b,
            "whi": wlayout(wT_hi), "wlo": wlayout(wT_lo),
            "multf": multf,
        })
    return in_maps


def kernel(inputs: np.ndarray, W: np.ndarray) -> np.ndarray:
    if "nc" not in _CACHE:
        _CACHE["nc"] = _build_program()
    nc = _CACHE["nc"]

    in_maps = _prep_inputs(np.asarray(inputs, np.float32), np.asarray(W, np.float32))

    kw = {}
    if os.environ.get("KERNEL_TRACE"):
        kw = {"trace": True}
    res = run_bass_kernel_spmd(nc, in_maps, core_ids=list(range(NCORES)), **kw)
    _CACHE["last_result"] = res
    if res.exec_time_ns is not None:
        print(f"HW exec time: {res.exec_time_ns} ns")

    full = np.empty((B, F_OUT, T), np.float32)
    for c in range(NCORES):
        a = res.results[c]["nspk"].reshape(P, M, NB, TL, B)
        a = a.transpose(4, 1, 0, 2, 3)             # [b, m, p, nb, tl]
        full[:, c * FS:(c + 1) * FS, :] = a.reshape(B, FS, T)
    return (np.float32(1.0) - full).astype(np.float32)


# revision 4
# speedup vs baseline: 1.0167x; 1.0167x over previous
"""Trainium2 Bass kernel for nn_Dense_25202868092933.

Computes: outs = einsum('bit,oi->bot', inputs, W); then a 64-step
leaky-integrate-and-fire neuron scan with hard reset:
    mem = mult*mem + scale*outs[..., t];  spk = (mem <= -0.5);  mem *= (1-spk)

Strategy (8 NeuronCores, no cross-core communication):
  - Tensor-parallel over out_features: each core owns 512 rows of W,
    the scale/mult vectors for those rows, and the scan state.
  - scale is folded into W on the host (W' = diag(scale) @ W), so the
    matmul directly produces scale*outs.
  - The matmul runs as a 3-pass bf16 hi/lo split (x_hi@W_hi + x_lo@W_hi
    + x_hi@W_lo), which reproduces fp32 accuracy to ~1.5e-6 abs on the
    pre-scan activations (measured: zero spike flips) at bf16 PE rate.
  - bt = t*64+b is the matmul moving dimension; blocks of 256 bt (4
    timesteps) pipeline matmul (PE) -> psum copy (ACT) -> scan (DVE)
    -> spike DMA, so the scan hides under the next block's matmuls.
  - The kernel emits "no-spike" indicators (mem > thr); the host
    computes spikes = 1 - nspk (exact complement of float 0/1).
"""

import os
import sys

for _p in ("/opt/trn_rl_repo", "/root/.axon_site", "/root/.axon_site/_ro/trn_rl_repo",
           "/root/.axon_site/_ro/pypackages"):
    if os.path.isdir(_p) and _p not in sys.path:
        sys.path.append(_p)

from contextlib import ExitStack

import ml_dtypes
import numpy as np

import concourse.bass as bass  # noqa: F401  (registers engine classes)
import concourse.mybir as mybir
import concourse.tile as tile
from concourse import bacc
from concourse.bass_utils import run_bass_kernel_spmd

# Problem constants
B, F_IN, F_OUT, T = 64, 4096, 4096, 64
KVCO = 5.0e8
KPD = 1.0 / np.pi
TIMESTEP = 1e-9
SCALING = KVCO * KPD * TIMESTEP / 2.0
THRESHOLD = -0.5

NCORES = 8
FS = F_OUT // NCORES      # 512 out-features per core
P = 128                   # partitions
M = FS // P               # 4 feature tiles per core
KO = F_IN // P            # 32 contraction tiles
BT = B * T                # 4096 moving dim (t-major: bt = t*B + b)
NJ = 256                  # bt per block
NB = BT // NJ             # 16 blocks
TL = NJ // B              # 4 timesteps per block

_dt = mybir.dt

_CACHE = {}


def _build_program():
    """Emit the SPMD Tile program (identical on all 8 cores)."""
    nc = bacc.Bacc("TRN2", target_bir_lowering=False, debug=False)

    xhi_d = nc.dram_tensor("xhi", [NB, P, KO * NJ], _dt.bfloat16, kind="ExternalInput").ap()
    xlo_d = nc.dram_tensor("xlo", [NB, P, KO * NJ], _dt.bfloat16, kind="ExternalInput").ap()
    whi_d = nc.dram_tensor("whi", [P, M * KO * P], _dt.bfloat16, kind="ExternalInput").ap()
    wlo_d = nc.dram_tensor("wlo", [P, M * KO * P], _dt.bfloat16, kind="ExternalInput").ap()
    mlt_d = nc.dram_tensor("multf", [P, M * B], _dt.float32, kind="ExternalInput").ap()
    out_d = nc.dram_tensor("nspk", [P, M * NB * NJ], _dt.float32, kind="ExternalOutput").ap()
    out4 = out_d.rearrange("p (m nb j) -> p m nb j", m=M, nb=NB)

    with tile.TileContext(nc) as tc, ExitStack() as ctx:
        wpool = ctx.enter_context(tc.tile_pool(name="wpool", bufs=1))
        cpool = ctx.enter_context(tc.tile_pool(name="cpool", bufs=1))
        xpool = ctx.enter_context(tc.tile_pool(name="xpool", bufs=2))
        ypool = ctx.enter_context(tc.tile_pool(name="ypool", bufs=2))
        npool = ctx.enter_context(tc.tile_pool(name="npool", bufs=2))
        spool = ctx.enter_context(tc.tile_pool(name="spool", bufs=1))
        tpool = ctx.enter_context(tc.tile_pool(name="tpool", bufs=2))
        pspool = ctx.enter_context(tc.tile_pool(name="pspool", bufs=8, space="PSUM"))

        whi4 = whi_d.rearrange("p (m k c) -> p m k c", m=M, k=KO)
        wlo4 = wlo_d.rearrange("p (m k c) -> p m k c", m=M, k=KO)
        # Startup-latency-critical ordering: first matmul needs w_hi[m=0]
        # and the first half of x block 0. Interleave across the two DMA
        # paths (sync=HWDGE, gpsimd=SWDGE) in ~1 MB chunks so the PE can
        # start ~6 us in instead of waiting ~40 us for serialized loads.
        w_hi = wpool.tile([P, M, KO, P], _dt.bfloat16, tag="whi", name="w_hi")
        w_lo = wpool.tile([P, M, KO, P], _dt.bfloat16, tag="wlo", name="w_lo")
        nc.sync.dma_start(w_hi[:, 0], whi4[:, 0])
        nc.gpsimd.dma_start(w_lo[:, 0], wlo4[:, 0])

        xh0 = xpool.tile([P, KO, NJ], _dt.bfloat16, tag="xh", name="xh0")
        xl0 = xpool.tile([P, KO, NJ], _dt.bfloat16, tag="xl", name="xl0")
        nc.sync.dma_start(xh0[:, 0:KO // 2, :], xhi_d[0].rearrange("p (k j) -> p k j", k=KO)[:, 0:KO // 2, :])
        nc.gpsimd.dma_start(xl0[:, 0:KO // 2, :], xlo_d[0].rearrange("p (k j) -> p k j", k=KO)[:, 0:KO // 2, :])
        nc.sync.dma_start(xh0[:, KO // 2:, :], xhi_d[0].rearrange("p (k j) -> p k j", k=KO)[:, KO // 2:, :])
        nc.gpsimd.dma_start(xl0[:, KO // 2:, :], xlo_d[0].rearrange("p (k j) -> p k j", k=KO)[:, KO // 2:, :])
        for m in range(1, M):
            nc.sync.dma_start(w_hi[:, m], whi4[:, m])
            nc.gpsimd.dma_start(w_lo[:, m], wlo4[:, m])

        mlt = cpool.tile([P, M, B], _dt.float32, tag="mlt", name="mlt")
        nc.gpsimd.dma_start(mlt[:], mlt_d.rearrange("p (m b) -> p m b", m=M))

        mem = spool.tile([P, M, B], _dt.float32, tag="mem", name="mem")
        nc.vector.memset(mem[:], 0.0)

        for nb in range(NB):
            if nb == 0:
                xh, xl = xh0, xl0
            else:
                xh = xpool.tile([P, KO, NJ], _dt.bfloat16, tag="xh", name="xh")
                xl = xpool.tile([P, KO, NJ], _dt.bfloat16, tag="xl", name="xl")
                xsrc = xhi_d[nb].rearrange("p (k j) -> p k j", k=KO)
                lsrc = xlo_d[nb].rearrange("p (k j) -> p k j", k=KO)
                nc.sync.dma_start(xh[:, 0:KO // 2, :], xsrc[:, 0:KO // 2, :])
                nc.gpsimd.dma_start(xl[:, 0:KO // 2, :], lsrc[:, 0:KO // 2, :])
                nc.sync.dma_start(xh[:, KO // 2:, :], xsrc[:, KO // 2:, :])
                nc.gpsimd.dma_start(xl[:, KO // 2:, :], lsrc[:, KO // 2:, :])

            y = ypool.tile([P, M, TL, B], _dt.float32, tag="y", name="y")
            for m in range(M):
                ps = pspool.tile([P, TL, B], _dt.float32, tag="ps", name="ps")
                for ko in range(KO):
                    # weight-reuse-friendly order: whi used twice in a row
                    nc.tensor.matmul(ps[:], w_hi[:, m, ko, :], xh[:, ko, :],
                                     start=(ko == 0), stop=False)
                    nc.tensor.matmul(ps[:], w_hi[:, m, ko, :], xl[:, ko, :],
                                     start=False, stop=False)
                    nc.tensor.matmul(ps[:], w_lo[:, m, ko, :], xh[:, ko, :],
                                     start=False, stop=(ko == KO - 1))
                nc.scalar.copy(y[:, m, :, :], ps[:])

            nsp = npool.tile([P, M, TL, B], _dt.float32, tag="nsp", name="nsp")
            for tl in range(TL):
                tmp = tpool.tile([P, M, B], _dt.float32, tag="tmp", name="tmp")
                nc.vector.tensor_mul(tmp[:], mem[:], mlt[:])
                nc.vector.tensor_add(mem[:], tmp[:], y[:, :, tl, :])
                nc.vector.tensor_scalar(nsp[:, :, tl, :], mem[:], THRESHOLD, None,
                                        mybir.AluOpType.is_gt)
                nc.vector.tensor_mul(mem[:], mem[:], nsp[:, :, tl, :])

            nc.gpsimd.dma_start(out4[:, :, nb, :],
                                nsp[:].rearrange("p m tl b -> p m (tl b)"))

    nc.compile()
    return nc


def _prep_inputs(inputs: np.ndarray, W: np.ndarray):
    """Host-side preprocessing: neuron constants, scale folding, hi/lo
    bf16 split, and per-core data layouts."""
    f32 = np.float32
    rs = W.sum(axis=1, dtype=np.float64).astype(f32)
    ncst = np.maximum((f32(1.0) + rs) / f32(SCALING), f32(0.0)).astype(f32)
    scale = (f32(-1.0) / (f32(1.0) + ncst)).astype(f32)
    mult = ((ncst - f32(1.0)) / (ncst + f32(1.0))).astype(f32)

    Ws = (W * scale[:, None]).astype(f32)          # fold scale into W rows
    wT = np.ascontiguousarray(Ws.T)                # [F_IN, F_OUT]
    wT_hi = wT.astype(ml_dtypes.bfloat16)
    wT_lo = (wT - wT_hi.astype(f32)).astype(ml_dtypes.bfloat16)

    # x_flat[i, t*B + b] = inputs[b, i, t]
    x_flat = np.ascontiguousarray(inputs.transpose(1, 2, 0)).reshape(F_IN, BT)
    x_hi = x_flat.astype(ml_dtypes.bfloat16)
    x_lo = (x_flat - x_hi.astype(f32)).astype(ml_dtypes.bfloat16)

    def xblocks(xa):
        # [F_IN, BT] -> [NB, P, KO*NJ]; per-partition contiguous 16 KB
        return np.ascontiguousarray(
            xa.reshape(KO, P, NB, NJ).transpose(2, 1, 0, 3)).reshape(NB, P, KO * NJ)

    xh_b = xblocks(x_hi)
    xl_b = xblocks(x_lo)

    in_maps = []
    for c in range(NCORES):
        sl = slice(c * FS, (c + 1) * FS)

        def wlayout(wa):
            # [F_IN, FS] -> [P, M*KO*P]: w[p, m, ko, cc] = wT[ko*P+p, m*P+cc]
            return np.ascontiguousarray(
                wa[:, sl].reshape(KO, P, M, P).transpose(1, 2, 0, 3)).reshape(P, M * KO * P)

        mc = mult[sl].reshape(M, P).T              # [P, M]
        multf = np.ascontiguousarray(
            np.broadcast_to(mc[:, :, None], (P, M, B))).reshape(P, M * B).astype(f32)

        in_maps.append({
            "xhi": xh_b, "xlo": xl_b,
            "whi": wlayout(wT_hi), "wlo": wlayout(wT_lo),
            "multf": multf,
        })
    return in_maps


def kernel(inputs: np.ndarray, W: np.ndarray) -> np.ndarray:
    if "nc" not in _CACHE:
        _CACHE["nc"] = _build_program()
    nc = _CACHE["nc"]

    in_maps = _prep_inputs(np.asarray(inputs, np.float32), np.asarray(W, np.float32))

    kw = {}
    if os.environ.get("KERNEL_TRACE"):
        kw = {"trace": True}
    res = run_bass_kernel_spmd(nc, in_maps, core_ids=list(range(NCORES)), **kw)
    _CACHE["last_result"] = res
    if res.exec_time_ns is not None:
        print(f"HW exec time: {res.exec_time_ns} ns")

    full = np.empty((B, F_OUT, T), np.float32)
    for c in range(NCORES):
        a = res.results[c]["nspk"].reshape(P, M, NB, TL, B)
        a = a.transpose(4, 1, 0, 2, 3)             # [b, m, p, nb, tl]
        full[:, c * FS:(c + 1) * FS, :] = a.reshape(B, FS, T)
    return (np.float32(1.0) - full).astype(np.float32)
